# revision 4
# baseline (speedup 1.0000x reference)
"""HQQ grouped (per-expert) int4-dequant GEMM on 8 trn2 NeuronCores.

Math per expert e (group g = k // group_size):
    W_e[k, n] = (q_e[k, n] - 8) * scale_e[g, n] + zero_e[g, n]
    out[rows_e] = x[rows_e] @ W_e          (rows_e contiguous, expert-sorted)

Sharding: 16 units = (expert, out-half).  Each core gets two units (slot A,
slot B) chosen so per-core token counts balance.  Host repacks int4 codes to
int8 and transposes x to bf16; the device casts q to bf16 (gpsimd cast-DMA),
multiplies by a partition-replicated scale tile (DVE tensor_tensor, 2x bf16)
and accumulates x^T.T @ (q*scale) over 16 K-tiles plus one extra K-tile that
applies the folded zero-point term  xs @ (zero - 8*scale)  where xs are the
per-group sums of x.
"""

import math
import os

import ml_dtypes
import numpy as np


def _ensure_ntff_hook():
    """The agent image's `antenv` lacks `axon_hooks`, so boot() skipped
    registering the NTFF profiling hook and trace=True would degrade to a
    no-profile run.  Recreate the module + register the ctypes hook."""
    import sys
    import types

    if "antenv.axon_hooks" in sys.modules:
        return
    try:
        import antenv  # noqa: F401

        mod = types.ModuleType("antenv.axon_hooks")
        mod._hook = None

        def set_axon_ntff_profile_hook(h):
            mod._hook = h

        def get_axon_ntff_profile_hook():
            return mod._hook

        mod.set_axon_ntff_profile_hook = set_axon_ntff_profile_hook
        mod.get_axon_ntff_profile_hook = get_axon_ntff_profile_hook
        sys.modules["antenv.axon_hooks"] = mod

        from trn_agent_boot.trn_boot import _ntff_profile_via_ctypes

        hook = _ntff_profile_via_ctypes("/opt/axon/libaxon_pjrt.so")
        if hook is not None:
            set_axon_ntff_profile_hook(hook)
    except Exception:
        pass


_ensure_ntff_hook()

E, T, IN, OUT = 8, 2048, 2048, 2048
P = 128
NCORES = 8
NHALF = OUT // 2  # 1024
MM_N = 512  # one PSUM bank
BF16 = ml_dtypes.bfloat16

_PROGRAM_CACHE = {}
LAST_RESULT = None


def _build_program(a_cap, b_cap, gs):
    """Build + compile the SPMD Bass program for slot capacities (in 128-token
    tiles) a_cap/b_cap and quant group size gs."""
    import concourse.bacc as bacc
    import concourse.mybir as mybir
    import concourse.tile as tile
    from contextlib import ExitStack

    bf16 = mybir.dt.bfloat16
    f32 = mybir.dt.float32
    i8 = mybir.dt.int8

    G = IN // gs       # quant groups (32)
    KT = IN // P       # 16 k-tiles
    RPT = P // gs      # groups spanned by one 128-row k-tile (2)

    nc = bacc.Bacc(
        "TRN2",
        target_bir_lowering=False,
        debug=False,
        enable_asserts=True,
        num_devices=NCORES,
    )

    slots = []
    for name, cap in (("a", a_cap), ("b", b_cap)):
        capT = cap * P
        slots.append(
            dict(
                name=name,
                cap=cap,
                capT=capT,
                q=nc.dram_tensor(f"q{name}", [IN, NHALF], i8, kind="ExternalInput").ap(),
                s=nc.dram_tensor(f"s{name}", [G, NHALF], bf16, kind="ExternalInput").ap(),
                z=nc.dram_tensor(f"z{name}", [P, NHALF], bf16, kind="ExternalInput").ap(),
                xt=nc.dram_tensor(f"x{name}", [IN, capT], bf16, kind="ExternalInput").ap(),
                xs=nc.dram_tensor(f"xs{name}", [P, capT], bf16, kind="ExternalInput").ap(),
                y=nc.dram_tensor(f"y{name}", [capT, NHALF], f32, kind="ExternalOutput").ap(),
            )
        )

    with tile.TileContext(nc) as tc, ExitStack() as ctx:
        xpool = ctx.enter_context(tc.tile_pool(name="x", bufs=1))
        wpool = ctx.enter_context(tc.tile_pool(name="w", bufs=2))
        qpool = ctx.enter_context(tc.tile_pool(name="q", bufs=4))
        rpool = ctx.enter_context(tc.tile_pool(name="rep", bufs=1))
        spool = ctx.enter_context(tc.tile_pool(name="s", bufs=2))
        opool = ctx.enter_context(tc.tile_pool(name="o", bufs=4))
        pspool = ctx.enter_context(tc.tile_pool(name="ps", bufs=4, space="PSUM"))

        for sl in slots:
            cap, capT = sl["cap"], sl["capT"]
            nm = sl["name"]

            x_sb = xpool.tile([P, KT, capT], bf16, tag=f"x{nm}")
            nc.sync.dma_start(x_sb[:], sl["xt"].rearrange("(kt p) t -> p kt t", p=P))
            xs_sb = xpool.tile([P, capT], bf16, tag=f"xs{nm}")
            nc.sync.dma_start(xs_sb[:], sl["xs"])
            z_sb = spool.tile([P, NHALF], bf16, tag="z")
            nc.sync.dma_start(z_sb[:], sl["z"])

            # replicated scales: srep[p, kt, n] = scale[RPT*kt + p//gs, n].
            # Load group-rows to partitions {0, gs, ...}, then double the
            # replica count 6x within each gs-partition half (all DMAs have
            # nonzero partition stride).
            srep = rpool.tile([P, KT, NHALF], bf16, tag="srep")
            nc.sync.dma_start(
                srep.rearrange("(j r) kt n -> j r kt n", j=RPT)[:, 0:1],
                sl["s"].rearrange("(kt j) n -> j kt n", j=RPT)[:, None],
            )
            for h in range(RPT):
                base = h * gs
                i = 1
                while i < gs:
                    nc.sync.dma_start(
                        srep[base + i : base + 2 * i], srep[base : base + i]
                    )
                    i *= 2

            w_sb = wpool.tile([P, KT, NHALF], bf16, tag="w")
            q3 = sl["q"].rearrange("(kt p) n -> p kt n", p=P)
            for kt in range(KT):
                qb = qpool.tile([P, NHALF], bf16, tag="qb")
                nc.gpsimd.dma_start(qb[:], q3[:, kt])  # cast i8 -> bf16
                nc.vector.tensor_tensor(
                    w_sb[:, kt, :], qb[:], srep[:, kt, :], mybir.AluOpType.mult
                )

            for tt in range(cap):
                for nb in range(NHALF // MM_N):
                    ps = pspool.tile([P, MM_N], f32, tag="ps")
                    for kt in range(KT):
                        nc.tensor.matmul(
                            ps[:],
                            x_sb[:, kt, tt * P : (tt + 1) * P],
                            w_sb[:, kt, nb * MM_N : (nb + 1) * MM_N],
                            start=(kt == 0),
                            stop=False,
                        )
                    nc.tensor.matmul(
                        ps[:],
                        xs_sb[:, tt * P : (tt + 1) * P],
                        z_sb[:, nb * MM_N : (nb + 1) * MM_N],
                        start=False,
                        stop=True,
                    )
                    o_sb = opool.tile([P, MM_N], f32, tag="o")
                    nc.any.tensor_copy(out=o_sb[:], in_=ps[:])
                    nc.sync.dma_start(
                        sl["y"][tt * P : (tt + 1) * P, nb * MM_N : (nb + 1) * MM_N],
                        o_sb[:],
                    )

    nc.compile()
    return nc


def _plan(tokens_per_expert):
    """Assign the 16 (expert, half) units to 8 cores x 2 slots."""
    tpe = np.asarray(tokens_per_expert).astype(np.int64)
    units = []
    for e in range(E):
        tt = int(math.ceil(tpe[e] / P))
        for h in range(2):
            units.append((tt, e, h))
    units.sort(key=lambda u: -u[0])
    a_units, b_units = units[:NCORES], units[NCORES:]
    # pair biggest A with smallest B for mild DMA smoothing
    b_units = b_units[::-1]
    a_cap = max(1, max(u[0] for u in a_units))
    b_cap = max(1, max(u[0] for u in b_units))
    return a_units, b_units, a_cap, b_cap


def kernel(x, qweight, scales_and_zeros, tokens_per_expert, group_size, **_):
    global LAST_RESULT
    from concourse.bass_utils import run_bass_kernel_spmd

    gs = int(group_size)
    G = IN // gs

    x = np.asarray(x, dtype=np.float32)
    qweight = np.asarray(qweight)
    snz = np.asarray(scales_and_zeros, dtype=np.float32)
    tpe = np.asarray(tokens_per_expert).astype(np.int64)
    bounds = np.concatenate([[0], np.cumsum(tpe)]).astype(np.int64)

    a_units, b_units, a_cap, b_cap = _plan(tpe)
    key = (a_cap, b_cap, gs)
    if key not in _PROGRAM_CACHE:
        _PROGRAM_CACHE[key] = _build_program(a_cap, b_cap, gs)
    nc = _PROGRAM_CACHE[key]

    # host-side layout prep (value-preserving repack/transpose/cast only)
    xT = np.ascontiguousarray(x.T).astype(BF16)                      # [IN, T]
    xs_all = np.ascontiguousarray(
        x.reshape(T, G, gs).sum(axis=2, dtype=np.float32).T
    ).astype(BF16)                                                   # [G, T]
    q8 = qweight.astype(np.int8)                                     # [E, IN, OUT]
    sc = snz[..., 0]                                                 # [E, G, OUT]
    zp = (snz[..., 1] - 8.0 * sc).astype(BF16)                       # zero' = zero-8*scale
    sc16 = sc.astype(BF16)

    in_maps = []
    for c in range(NCORES):
        m = {}
        for slot, cap, (tt, e, h) in (("a", a_cap, a_units[c]), ("b", b_cap, b_units[c])):
            capT = cap * P
            r0, r1 = int(bounds[e]), int(bounds[e + 1])
            n0, n1 = h * NHALF, (h + 1) * NHALF
            xa = np.zeros([IN, capT], BF16)
            xa[:, : r1 - r0] = xT[:, r0:r1]
            xsa = np.zeros([P, capT], BF16)
            xsa[:G, : r1 - r0] = xs_all[:, r0:r1]
            za = np.zeros([P, NHALF], BF16)
            za[:G] = zp[e, :, n0:n1]
            m[f"x{slot}"] = xa
            m[f"xs{slot}"] = xsa
            m[f"q{slot}"] = np.ascontiguousarray(q8[e, :, n0:n1])
            m[f"s{slot}"] = np.ascontiguousarray(sc16[e, :, n0:n1])
            m[f"z{slot}"] = za
        in_maps.append(m)

    res = run_bass_kernel_spmd(nc, in_maps, list(range(NCORES)))
    LAST_RESULT = res

    out = np.zeros([T, OUT], np.float32)
    for c in range(NCORES):
        for slot, (tt, e, h) in (("a", a_units[c]), ("b", b_units[c])):
            r0, r1 = int(bounds[e]), int(bounds[e + 1])
            out[r0:r1, h * NHALF : (h + 1) * NHALF] = res.results[c][f"y{slot}"][
                : r1 - r0
            ]
    return out
